# revision 15
# baseline (speedup 1.0000x reference)
"""Trainium2 Bass kernel for nn_GateCircuit (14-qubit batched gate circuit).

Math: the reference applies RX(x@W.T[:,i]) then RY(params[i]) on wire i of
|0...0> (a product state stays a product state since each gate hits a distinct
wire), then a CNOT ladder CNOT(i, i+1), then measures <Z_0>.  Qubit 0 is only
ever a CNOT *control*, so its marginal is untouched by the ladder; the
expectation collapses to the single-qubit value

    <Z_0> = cos(x @ W[0]) * cos(params[0])
    out   = sigmoid(<Z_0>)

Sharding: pure data parallel, batch 4096 split 512 per core across 8 cores;
W row 0 and params[0] shipped as one [1,257] row and replicated across the
128 SBUF partitions by a partition-broadcast DMA (0-stride source AP).

No activation engine at all: sigmoid is a degree-2 odd polynomial on DVE
(err 1.3e-5), so there are no act-table loads -- the scalar engine runs
only DMA injects and its HWDGE ring streams at full rate.

Ring budget (HWDGE rings expand ~1 descriptor per ~10ns; every [128,*]
transfer costs 128 descriptors, so each ring gets at most two of them):
  scalar ring:  wp broadcast (first), output store partitions 0..63
  sync ring:    x half A [128 x 2KB lines], output store partitions 64..127
  gpsimd SWDGE: x half B [128 x 2KB lines] (injected first on gpsimd)

On-device per core (all f32):
  z[:, n] = sum_f (x*inv2pi) * w            4x DVE STT with accumulator;
                                            1/2pi folded in, z in periods
  k = int(z)                                f32->i32 cast rounds to nearest
                                            on HW (verified on device)
  f = k - z in [-0.5, 0.5]                  one STT; sign dies in v = f^2
  P(v) = C0+C1 v+C2 v^2+C3 v^3 ~= cos(2pi f) = cos(x@W[0]), err 1.4e-3
  a = Pp*q3 + Pb = cos(p0)*cos(x@W[0])      Pp = P(v_p) = cos(p0), Pb=Pp*C0,
                                            computed on gpsimd off-path
  out = 0.5 + a*(E0 + E1 u + E2 u^2), u=a^2 degree-2 odd sigmoid on DVE
"""

import math

import numpy as np

_NCORES = 8
_B = 4096
_F = 256
_BS = _B // _NCORES  # 512 samples per core
_NT = _BS // 128     # 4 sample-blocks per partition
_INV_TWO_PI = float(1.0 / (2.0 * math.pi))

# P(v) = C0 + C1 v + C2 v^2 + C3 v^3 ~= cos(2pi f), v = f^2, f in [-.5, .5]
_C0 = 0.9985678609910458
_C1 = -19.552759014070162
_C2 = 61.10740166704636
_C3 = -59.580321884808846
# sigmoid(a) = 0.5 + a*(E0 + E1 u + E2 u^2), u = a^2, a in [-1.01, 1.01]
_E0 = 0.24999587
_E1 = -0.02074685
_E2 = 0.00181964

_CACHE: dict = {}


def _build():
    import concourse.bacc as bacc
    import concourse.mybir as mybir
    import concourse.tile as tile

    f32 = mybir.dt.float32
    i32 = mybir.dt.int32
    Alu = mybir.AluOpType

    nc = bacc.Bacc("TRN2", target_bir_lowering=False, debug=False,
                   num_devices=_NCORES)

    x_d = nc.dram_tensor("x", [_BS, _F], f32, kind="ExternalInput")
    wp_d = nc.dram_tensor("wp", [128, _F + 1], f32, kind="ExternalInput")
    o_d = nc.dram_tensor("o", [_BS], f32, kind="ExternalOutput")

    with tile.TileContext(nc) as tc:
        with (
            tc.tile_pool(name="xin", bufs=1) as xpool,
            tc.tile_pool(name="scratch", bufs=2) as spool,
            tc.tile_pool(name="small", bufs=1) as zpool,
        ):
            # --- input DMAs.  wp first on the sync ring (fastest startup);
            # x block 0 on the scalar ring, blocks 1-3 on the SWDGE ring
            # (which measured fastest for bulk), each with its own
            # semaphore so dots chase arrivals. ---
            wb = zpool.tile([128, _F + 1], f32)
            nc.sync.dma_start(wb[:], wp_d[:, :])
            xr = x_d.ap().rearrange("(p n) f -> p (n f)", n=_NT)  # [128,1024]
            xt = xpool.tile([128, _NT * _F], f32)
            nc.gpsimd.dma_start(xt[:, _F:], xr[:, _F:])
            nc.scalar.dma_start(xt[:, 0:_F], xr[:, 0:_F])

            # --- params chain on gpsimd: Pp = P(frac(p0/2pi)^2) = cos(p0),
            #     Pb = Pp*C0.  [128,1] native ops, off the DVE path. ---
            pz = zpool.tile([128, 1], f32)
            nc.gpsimd.tensor_scalar_mul(pz[:], wb[:, _F:_F + 1], _INV_TWO_PI)
            pk = zpool.tile([128, 1], i32)
            nc.gpsimd.tensor_copy(pk[:], pz[:])
            pkf = zpool.tile([128, 1], f32)
            nc.gpsimd.tensor_copy(pkf[:], pk[:])
            pd = zpool.tile([128, 1], f32)
            nc.gpsimd.tensor_tensor(pd[:], pz[:], pkf[:], op=Alu.subtract)
            pv = zpool.tile([128, 1], f32)
            nc.gpsimd.tensor_tensor(pv[:], pd[:], pd[:], op=Alu.mult)
            ps1 = zpool.tile([128, 1], f32)
            nc.gpsimd.tensor_scalar(ps1[:], pv[:], _C3, _C2,
                                    op0=Alu.mult, op1=Alu.add)
            pm1 = zpool.tile([128, 1], f32)
            nc.gpsimd.tensor_tensor(pm1[:], ps1[:], pv[:], op=Alu.mult)
            ps2 = zpool.tile([128, 1], f32)
            nc.gpsimd.tensor_scalar(ps2[:], pm1[:], _C1, 1.0,
                                    op0=Alu.add, op1=Alu.mult)
            ps3 = zpool.tile([128, 1], f32)
            nc.gpsimd.tensor_tensor(ps3[:], ps2[:], pv[:], op=Alu.mult)

            # --- dot products z[:, n] = sum_f x_blk_n*inv2pi * w  (DVE) ---
            w256 = wb[:, 0:_F]
            z = zpool.tile([128, _NT], f32)
            for n in range(_NT):
                prod = spool.tile([128, _F], f32)
                nc.vector.scalar_tensor_tensor(
                    prod[:], xt[:, n * _F:(n + 1) * _F], _INV_TWO_PI, w256,
                    op0=Alu.mult, op1=Alu.mult,
                    accum_out=z[:, n:n + 1],
                )

            # --- range reduce + cos poly (DVE): q3 = P(v) - C0 ---
            k = zpool.tile([128, _NT], i32)
            nc.vector.tensor_copy(k[:], z[:])
            kf = zpool.tile([128, _NT], f32)
            nc.vector.tensor_copy(kf[:], k[:])
            f = zpool.tile([128, _NT], f32)
            nc.vector.scalar_tensor_tensor(f[:], kf[:], 0.0, z[:],
                                           op0=Alu.bypass, op1=Alu.subtract)
            v = zpool.tile([128, _NT], f32)
            nc.vector.tensor_tensor(v[:], f[:], f[:], op=Alu.mult)
            q1 = zpool.tile([128, _NT], f32)
            nc.vector.tensor_scalar(q1[:], v[:], _C3, _C2,
                                    op0=Alu.mult, op1=Alu.add)
            q2 = zpool.tile([128, _NT], f32)
            nc.vector.scalar_tensor_tensor(q2[:], q1[:], 0.0, v[:],
                                           op0=Alu.bypass, op1=Alu.mult)
            q3 = zpool.tile([128, _NT], f32)
            nc.vector.scalar_tensor_tensor(q3[:], q2[:], _C1, v[:],
                                           op0=Alu.add, op1=Alu.mult)

            # --- Pp = ps3 + C0, Pb = Pp*C0 on DVE (the slow gpsimd chain
            # ends before the DVE tail does; these two land right after) ---
            pp = zpool.tile([128, 1], f32)
            nc.vector.tensor_scalar(pp[:], ps3[:], _C0, 1.0,
                                    op0=Alu.add, op1=Alu.mult)
            pb = zpool.tile([128, 1], f32)
            nc.vector.tensor_scalar(pb[:], ps3[:], _C0, _C0,
                                    op0=Alu.add, op1=Alu.mult)

            # --- a = Pp*q3 + Pb;  out = 0.5 + a*(E0 + E1 u + E2 u^2) ---
            a = zpool.tile([128, _NT], f32)
            nc.vector.tensor_scalar(a[:], q3[:], pp[:, :], pb[:, :],
                                    op0=Alu.mult, op1=Alu.add)
            u = zpool.tile([128, _NT], f32)
            nc.vector.tensor_tensor(u[:], a[:], a[:], op=Alu.mult)
            h1 = zpool.tile([128, _NT], f32)
            nc.vector.tensor_scalar(h1[:], u[:], _E2, _E1,
                                    op0=Alu.mult, op1=Alu.add)
            h2 = zpool.tile([128, _NT], f32)
            nc.vector.scalar_tensor_tensor(h2[:], h1[:], 0.0, u[:],
                                           op0=Alu.bypass, op1=Alu.mult)
            h3 = zpool.tile([128, _NT], f32)
            nc.vector.scalar_tensor_tensor(h3[:], h2[:], _E0, a[:],
                                           op0=Alu.add, op1=Alu.mult)
            ot = zpool.tile([128, _NT], f32)
            nc.vector.tensor_scalar(ot[:], h3[:], 1.0, 0.5,
                                    op0=Alu.mult, op1=Alu.add)

            # --- output store, split across the two HWDGE rings ---
            orr = o_d.ap().rearrange("(p n) -> p n", n=_NT)
            nc.scalar.dma_start(orr[0:64], ot[0:64, :])
            nc.sync.dma_start(orr[64:128], ot[64:128, :])

    nc.compile()
    return nc


def _get_nc():
    if "nc" not in _CACHE:
        _CACHE["nc"] = _build()
    return _CACHE["nc"]


def _in_maps(x, W, params):
    x = np.ascontiguousarray(np.asarray(x, dtype=np.float32))
    W = np.asarray(W, dtype=np.float32)
    params = np.asarray(params, dtype=np.float32)
    wp_row = np.concatenate([W[0], params[0:1]]).astype(np.float32)
    wp = np.ascontiguousarray(np.broadcast_to(wp_row, (128, _F + 1)))
    return [
        {"x": x[c * _BS:(c + 1) * _BS], "wp": wp}
        for c in range(_NCORES)
    ]


def run_spmd(x, W, params, **kw):
    """Compile (cached) and run on 8 cores; returns BassKernelResults.

    Retries a few times: the axon-relayed device occasionally reports a
    transient NRT_EXEC_UNIT_UNRECOVERABLE that clears on the next attempt.
    """
    import time

    from concourse import bass_utils

    nc = _get_nc()
    in_maps = _in_maps(x, W, params)
    last = None
    for attempt in range(4):
        try:
            return bass_utils.run_bass_kernel_spmd(
                nc, in_maps, list(range(_NCORES)), **kw
            )
        except Exception as e:  # transient device/relay errors
            last = e
            time.sleep(2.0 * (attempt + 1))
    raise last


def kernel(x, W, params):
    res = run_spmd(x, W, params)
    return np.concatenate([res.results[c]["o"] for c in range(_NCORES)], axis=0)
